# revision 1
# baseline (speedup 1.0000x reference)
"""Trainium2 Bass kernel for a 4-layer dense MLP (H=8192), batch=1.

Tensor-parallel over 8 NeuronCores:
  - Hidden-layer weights (8192x8192) are column-sharded: core c owns columns
    [c*1024, (c+1)*1024). Each core streams its fp16 weight shard from HBM and
    computes a 1024-wide slice of the next activation, then an AllGather
    rebuilds the full 8192-activation on every core.
  - Layer 1 (10x8192) is column-sharded the same way (tiny).
  - The output layer (8192x8) is row-sharded so no AllGather is needed after
    layer 4; each core emits a partial [8] which the host sums.

Compute dtype is fp16 (PSUM accumulation is fp32). Measured end-to-end
quantization error vs the f32 reference is ~3e-4 max-rel.

Weight layout is pre-arranged on the host so every weight DMA is a contiguous
1 MiB block landing as [128, 4096] SBUF tiles:
  - activations live in SBUF as [128, 64] with a[p, k] = a_full[p*64 + k]
    (so the AllGather output, which is rank-concatenated in natural order,
    reloads with a plain contiguous DMA)
  - weight chunk k therefore carries rows {p*64 + k : p in 0..127}.
"""

import numpy as np

H = 8192
D = 10  # input layer size (4 + 6)
OUT = 8
NCORES = 8
SH = H // NCORES  # 1024 columns per core
KC = 64  # contraction chunks of 128 rows per hidden layer
GW = 4  # chunks per DMA group (1 MiB per DMA)
G = KC // GW  # 16 groups
WBUFS = 8  # in-flight weight DMA buffers (8 MiB SBUF)

LAST_RESULTS = None
_CACHE = {}


def _build_nc():
    import concourse.bacc as bacc
    import concourse.mybir as mybir
    import concourse.tile as tile

    f16 = mybir.dt.float16
    f32 = mybir.dt.float32
    SIG = mybir.ActivationFunctionType.Sigmoid
    RG = [list(range(NCORES))]

    nc = bacc.Bacc(
        "TRN2", target_bir_lowering=False, debug=False, num_devices=NCORES
    )

    x_d = nc.dram_tensor("x_cat", [D, 1], f16, kind="ExternalInput")
    win_d = nc.dram_tensor("w_in", [D, SH], f16, kind="ExternalInput")
    whh_d = nc.dram_tensor("w_hh", [3, G, 128, GW * SH], f16, kind="ExternalInput")
    wout_d = nc.dram_tensor("w_out", [128, 8 * OUT], f16, kind="ExternalInput")
    bias_d = nc.dram_tensor("bias", [1, 4 * SH], f16, kind="ExternalInput")
    out_d = nc.dram_tensor("out_partial", [1, OUT], f32, kind="ExternalOutput")

    with tile.TileContext(nc) as tc:
        with (
            tc.tile_pool(name="const", bufs=1) as cp,
            tc.tile_pool(name="wpool", bufs=WBUFS) as wp,
            tc.tile_pool(name="apool", bufs=2) as ap,
            tc.tile_pool(name="pspool", bufs=2, space="PSUM") as pp,
            tc.tile_pool(name="dpool", bufs=2, space="DRAM") as dp,
        ):
            x_sb = cp.tile([D, 1], f16)
            nc.scalar.dma_start(x_sb[:], x_d[:])
            win_sb = cp.tile([D, SH], f16)
            nc.scalar.dma_start(win_sb[:], win_d[:])
            bias_sb = cp.tile([1, 4 * SH], f16)
            nc.scalar.dma_start(bias_sb[:], bias_d[:])
            wout_sb = cp.tile([128, 8 * OUT], f16)
            nc.scalar.dma_start(wout_sb[:], wout_d[:])
            one_sb = cp.tile([1, 1], f16)
            nc.gpsimd.memset(one_sb[:], 1.0)

            a_sb = None
            act_sb = None
            for li in range(4):
                ps = pp.tile([1, SH], f32, tag="ps")
                if li == 0:
                    for j in range(2):
                        nc.tensor.matmul(
                            ps[:, j * 512 : (j + 1) * 512],
                            x_sb[:],
                            win_sb[:, j * 512 : (j + 1) * 512],
                            start=True,
                            stop=False,
                        )
                else:
                    for g in range(G):
                        wt = wp.tile([128, GW * SH], f16, tag="w")
                        nc.sync.dma_start(wt[:], whh_d[li - 1, g])
                        for c in range(GW):
                            k = g * GW + c
                            for j in range(2):
                                nc.tensor.matmul(
                                    ps[:, j * 512 : (j + 1) * 512],
                                    a_sb[:, k : k + 1],
                                    wt[:, c * SH + j * 512 : c * SH + (j + 1) * 512],
                                    start=(k == 0),
                                    stop=False,
                                )
                # bias as a K=1 matmul closing the accumulation group
                for j in range(2):
                    nc.tensor.matmul(
                        ps[:, j * 512 : (j + 1) * 512],
                        one_sb[:],
                        bias_sb[:, li * SH + j * 512 : li * SH + (j + 1) * 512],
                        start=False,
                        stop=True,
                    )
                act_sb = ap.tile([1, SH], f16, tag="act")
                nc.scalar.activation(act_sb[:], ps[:], SIG)
                if li < 3:
                    cc_in = dp.tile([1, SH], f16, tag="ccin")
                    cc_out = dp.tile([128, KC], f16, tag="ccout")
                    nc.scalar.dma_start(cc_in[:], act_sb[:])
                    nc.gpsimd.collective_compute(
                        "AllGather",
                        mybir.AluOpType.bypass,
                        replica_groups=RG,
                        ins=[cc_in.opt()],
                        outs=[cc_out.opt()],
                    )
                    a_sb = ap.tile([128, KC], f16, tag="a")
                    nc.scalar.dma_start(a_sb[:], cc_out[:])

            # Output layer: transpose a4 shard into [128, 8] via a DRAM bounce,
            # then 8 accumulating matmuls against the row-sharded W_out.
            sc = dp.tile([1, SH], f16, tag="sc")
            nc.scalar.dma_start(sc[:], act_sb[:])
            a2_sb = ap.tile([128, 8], f16, tag="a2")
            nc.scalar.dma_start(
                a2_sb[:], sc.rearrange("one (p k) -> (one p) k", p=128)
            )
            pso = pp.tile([1, OUT], f32, tag="pso")
            for k in range(8):
                nc.tensor.matmul(
                    pso[:],
                    a2_sb[:, k : k + 1],
                    wout_sb[:, k * OUT : (k + 1) * OUT],
                    start=(k == 0),
                    stop=(k == 7),
                )
            res_sb = ap.tile([1, OUT], f32, tag="res")
            nc.vector.tensor_copy(res_sb[:], pso[:])
            nc.scalar.dma_start(out_d[:], res_sb[:])

    nc.compile()
    return nc


def _prep_inputs(x, s, W_in, W_hh, W_out, b):
    """Shard + fp16-quantize + lay out the inputs for each of the 8 cores."""
    f16 = np.float16
    x_cat = np.concatenate([np.asarray(x), np.asarray(s)]).astype(f16)
    x_cat = np.ascontiguousarray(x_cat.reshape(D, 1))
    Whh16 = np.asarray(W_hh).astype(f16)  # [3, 8192, 8192]
    Win16 = np.asarray(W_in).astype(f16)  # [10, 8192]
    Wout16 = np.asarray(W_out).astype(f16)  # [8192, 8]
    b16 = np.asarray(b).astype(f16)  # [5, 8192] (b[4] unused)

    in_maps = []
    for c in range(NCORES):
        cs, ce = c * SH, (c + 1) * SH
        # [8192, 1024] -> [64, 128, 1024] with chunk k holding rows p*64+k,
        # then 4 chunks per 1 MiB group -> [16, 128, 4096]
        shard = Whh16[:, :, cs:ce].reshape(3, 128, KC, SH)
        shard = shard.transpose(0, 2, 1, 3)  # [3, 64, 128, 1024]
        shard = shard.reshape(3, G, GW, 128, SH).transpose(0, 1, 3, 2, 4)
        whh_c = np.ascontiguousarray(shard.reshape(3, G, 128, GW * SH))
        wout_c = np.ascontiguousarray(
            Wout16[cs:ce, :].reshape(128, 8, OUT).reshape(128, 8 * OUT)
        )
        in_maps.append(
            {
                "x_cat": x_cat,
                "w_in": np.ascontiguousarray(Win16[:, cs:ce]),
                "w_hh": whh_c,
                "w_out": wout_c,
                "bias": np.ascontiguousarray(b16[0:4, cs:ce].reshape(1, 4 * SH)),
            }
        )
    return in_maps


def kernel(**inputs):
    global LAST_RESULTS
    import os

    from concourse import bass_utils

    if "nc" not in _CACHE:
        _CACHE["nc"] = _build_nc()
    nc = _CACHE["nc"]

    in_maps = _prep_inputs(**inputs)
    trace = bool(int(os.environ.get("BASS_TRACE_KERNEL", "0")))
    res = bass_utils.run_bass_kernel_spmd(
        nc, in_maps, core_ids=list(range(NCORES)), trace=trace
    )
    LAST_RESULTS = res
    partials = np.stack([r["out_partial"][0] for r in res.results])  # [8, 8]
    return partials.sum(axis=0).astype(np.float32)


# revision 5
# speedup vs baseline: 1.0838x; 1.0838x over previous
"""Trainium2 Bass kernel for a 4-layer dense MLP (H=8192), batch=1.

Tensor-parallel over 8 NeuronCores:
  - Layer 1 (10x8192) is replicated: every core computes the full a1 (cheap),
    so no collective is needed before layer 2.
  - Hidden-layer weights (8192x8192) are column-sharded: core c owns columns
    [c*1024, (c+1)*1024). Each core streams its fp16 weight shard from HBM,
    computes a 1024-wide slice of the next activation, then an AllGather
    rebuilds the full 8192-activation on every core (2 AllGathers total).
  - The output layer (8192x8) is row-sharded so no AllGather is needed after
    layer 4; each core emits a partial [8] which the host sums.
  - A dummy AllGather fires at kernel start to absorb the one-time ncfw
    rendezvous barrier (~26us) while weight streaming warms up.

Compute dtype is fp16 (PSUM accumulation is fp32); measured end-to-end error
vs the f32 reference is ~4e-4 max-rel.

Weight layout is pre-arranged on the host so every weight DMA is a contiguous
1 MiB block landing as [128, 4096] SBUF tiles:
  - activations live in SBUF as [128, 64] with a[p, k] = a_full[p*64 + k]
    (the AllGather output is rank-concatenated in natural order, so it
    reloads with a plain contiguous DMA)
  - weight chunk k therefore carries rows {p*64 + k : p in 0..127}.
"""

import numpy as np

H = 8192
D = 10  # input layer size (4 + 6)
OUT = 8
NCORES = 8
SH = H // NCORES  # 1024 columns per core
KC = 64  # contraction chunks of 128 rows per hidden layer
GW = 4  # chunks per DMA group (1 MiB per DMA)
G = KC // GW  # 16 groups
WBUFS = 16  # in-flight weight DMA buffers (16 MiB SBUF)

LAST_RESULTS = None
_CACHE = {}


def _build_nc():
    import concourse.bacc as bacc
    import concourse.mybir as mybir
    import concourse.tile as tile

    f16 = mybir.dt.float16
    f32 = mybir.dt.float32
    SIG = mybir.ActivationFunctionType.Sigmoid
    RG = [list(range(NCORES))]

    nc = bacc.Bacc(
        "TRN2", target_bir_lowering=False, debug=False, num_devices=NCORES
    )

    x_d = nc.dram_tensor("x_cat", [D, 1], f16, kind="ExternalInput")
    win_d = nc.dram_tensor("w_in", [D, H], f16, kind="ExternalInput")
    whh_d = nc.dram_tensor("w_hh", [3, G, 128, GW * SH], f16, kind="ExternalInput")
    wout_d = nc.dram_tensor("w_out", [128, 8 * OUT], f16, kind="ExternalInput")
    bias0_d = nc.dram_tensor("bias0", [1, H], f16, kind="ExternalInput")
    bias_d = nc.dram_tensor("bias", [1, 3 * SH], f16, kind="ExternalInput")
    out_d = nc.dram_tensor("out_partial", [1, OUT], f32, kind="ExternalOutput")

    with tile.TileContext(nc) as tc:
        with (
            tc.tile_pool(name="const", bufs=1) as cp,
            tc.tile_pool(name="wpool", bufs=WBUFS) as wp,
            tc.tile_pool(name="apool", bufs=2) as ap,
            tc.tile_pool(name="pspool", bufs=2, space="PSUM") as pp,
            tc.tile_pool(name="dpool", bufs=2, space="DRAM") as dp,
        ):
            one_sb = cp.tile([1, 1], f16)
            nc.gpsimd.memset(one_sb[:], 1.0)

            # Dummy collective first: absorbs the one-time ncfw rendezvous
            # barrier concurrently with layer-1 compute + weight prefetch.
            warm_sb = cp.tile([1, 16], f16)
            nc.gpsimd.memset(warm_sb[:], 0.0)
            warm_in = dp.tile([1, 16], f16, tag="warmin")
            warm_out = dp.tile([8, 16], f16, tag="warmout")
            nc.gpsimd.dma_start(warm_in[:], warm_sb[:])
            nc.gpsimd.collective_compute(
                "AllGather",
                mybir.AluOpType.bypass,
                replica_groups=RG,
                ins=[warm_in.opt()],
                outs=[warm_out.opt()],
            )

            x_sb = cp.tile([D, 1], f16)
            nc.scalar.dma_start(x_sb[:], x_d[:])
            win_sb = cp.tile([D, H], f16)
            nc.scalar.dma_start(win_sb[:], win_d[:])
            bias0_sb = cp.tile([1, H], f16)
            nc.scalar.dma_start(bias0_sb[:], bias0_d[:])
            bias_sb = cp.tile([1, 3 * SH], f16)
            nc.scalar.dma_start(bias_sb[:], bias_d[:])
            wout_sb = cp.tile([128, 8 * OUT], f16)
            nc.scalar.dma_start(wout_sb[:], wout_d[:])

            # ---- Layer 1, replicated: full a1 [1, 8192] in 8 psum passes ----
            act1_sb = cp.tile([1, H], f16)
            for h in range(8):  # 1024 columns per pass
                ps1 = pp.tile([1, SH], f32, tag="ps")
                for j in range(2):
                    lo = j * 512
                    nc.tensor.matmul(
                        ps1[:, lo : lo + 512],
                        x_sb[:],
                        win_sb[:, h * SH + lo : h * SH + lo + 512],
                        start=True,
                        stop=False,
                    )
                    nc.tensor.matmul(
                        ps1[:, lo : lo + 512],
                        one_sb[:],
                        bias0_sb[:, h * SH + lo : h * SH + lo + 512],
                        start=False,
                        stop=True,
                    )
                nc.scalar.activation(
                    act1_sb[:, h * SH : (h + 1) * SH], ps1[:], SIG
                )
            a1_d = dp.tile([1, H], f16, tag="a1")
            nc.scalar.dma_start(a1_d[:], act1_sb[:])
            a_sb = ap.tile([128, KC], f16, tag="a")
            nc.scalar.dma_start(
                a_sb[:], a1_d.rearrange("one (p k) -> (one p) k", p=128)
            )

            # ---- Hidden layers 2-4: column-sharded, AllGather between ----
            act_sb = None
            for li in range(3):
                ps = pp.tile([1, SH], f32, tag="ps")
                for g in range(G):
                    wt = wp.tile([128, GW * SH], f16, tag="w")
                    nc.sync.dma_start(wt[:], whh_d[li, g])
                    for c in range(GW):
                        k = g * GW + c
                        for j in range(2):
                            nc.tensor.matmul(
                                ps[:, j * 512 : (j + 1) * 512],
                                a_sb[:, k : k + 1],
                                wt[:, c * SH + j * 512 : c * SH + (j + 1) * 512],
                                start=(k == 0),
                                stop=False,
                            )
                # bias as a K=1 matmul closing the accumulation group
                for j in range(2):
                    nc.tensor.matmul(
                        ps[:, j * 512 : (j + 1) * 512],
                        one_sb[:],
                        bias_sb[:, li * SH + j * 512 : li * SH + (j + 1) * 512],
                        start=False,
                        stop=True,
                    )
                act_sb = ap.tile([1, SH], f16, tag="act")
                nc.scalar.activation(act_sb[:], ps[:], SIG)
                if li < 2:
                    cc_in = dp.tile([1, SH], f16, tag="ccin")
                    cc_out = dp.tile([128, KC], f16, tag="ccout")
                    nc.gpsimd.dma_start(cc_in[:], act_sb[:])
                    nc.gpsimd.collective_compute(
                        "AllGather",
                        mybir.AluOpType.bypass,
                        replica_groups=RG,
                        ins=[cc_in.opt()],
                        outs=[cc_out.opt()],
                    )
                    a_sb = ap.tile([128, KC], f16, tag="a")
                    nc.scalar.dma_start(a_sb[:], cc_out[:])

            # ---- Output layer: row-sharded, partial [8] per core ----
            sc = dp.tile([1, SH], f16, tag="sc")
            nc.scalar.dma_start(sc[:], act_sb[:])
            a2_sb = ap.tile([128, 8], f16, tag="a2")
            nc.scalar.dma_start(
                a2_sb[:], sc.rearrange("one (p k) -> (one p) k", p=128)
            )
            pso = pp.tile([1, OUT], f32, tag="pso")
            for k in range(8):
                nc.tensor.matmul(
                    pso[:],
                    a2_sb[:, k : k + 1],
                    wout_sb[:, k * OUT : (k + 1) * OUT],
                    start=(k == 0),
                    stop=(k == 7),
                )
            res_sb = ap.tile([1, OUT], f32, tag="res")
            nc.vector.tensor_copy(res_sb[:], pso[:])
            nc.scalar.dma_start(out_d[:], res_sb[:])

    nc.compile()
    return nc


def _prep_inputs(x, s, W_in, W_hh, W_out, b):
    """Shard + fp16-quantize + lay out the inputs for each of the 8 cores."""
    f16 = np.float16
    x_cat = np.concatenate([np.asarray(x), np.asarray(s)]).astype(f16)
    x_cat = np.ascontiguousarray(x_cat.reshape(D, 1))
    Whh16 = np.asarray(W_hh).astype(f16)  # [3, 8192, 8192]
    Win16 = np.ascontiguousarray(np.asarray(W_in).astype(f16))  # [10, 8192]
    Wout16 = np.asarray(W_out).astype(f16)  # [8192, 8]
    b16 = np.asarray(b).astype(f16)  # [5, 8192] (b[4] unused)
    bias0 = np.ascontiguousarray(b16[0].reshape(1, H))

    in_maps = []
    for c in range(NCORES):
        cs, ce = c * SH, (c + 1) * SH
        # [8192, 1024] -> chunk k holds rows p*64+k -> 1 MiB groups of 4 chunks
        shard = Whh16[:, :, cs:ce].reshape(3, 128, KC, SH)
        shard = shard.transpose(0, 2, 1, 3)  # [3, 64, 128, 1024]
        shard = shard.reshape(3, G, GW, 128, SH).transpose(0, 1, 3, 2, 4)
        whh_c = np.ascontiguousarray(shard.reshape(3, G, 128, GW * SH))
        wout_c = np.ascontiguousarray(Wout16[cs:ce, :].reshape(128, 8 * OUT))
        in_maps.append(
            {
                "x_cat": x_cat,
                "w_in": Win16,
                "w_hh": whh_c,
                "w_out": wout_c,
                "bias0": bias0,
                "bias": np.ascontiguousarray(b16[1:4, cs:ce].reshape(1, 3 * SH)),
            }
        )
    return in_maps


def kernel(**inputs):
    global LAST_RESULTS
    import os

    from concourse import bass_utils

    if "nc" not in _CACHE:
        _CACHE["nc"] = _build_nc()
    nc = _CACHE["nc"]

    in_maps = _prep_inputs(**inputs)
    trace = bool(int(os.environ.get("BASS_TRACE_KERNEL", "0")))
    res = bass_utils.run_bass_kernel_spmd(
        nc, in_maps, core_ids=list(range(NCORES)), trace=trace
    )
    LAST_RESULTS = res
    partials = np.stack([r["out_partial"][0] for r in res.results])  # [8, 8]
    return partials.sum(axis=0).astype(np.float32)
